# revision 67
# baseline (speedup 1.0000x reference)
"""Trainium2 Bass kernel for nn_Block_with_lora (dense transformer block).

Sharding: 8 cores = 4 batches x 2 token-parity shards (stride-2 over T).
Each core computes its 512 query tokens end-to-end (no collectives);
K/V projections over all 1024 tokens are computed per-core.

Host-side prep folds LoRA (W + s*B*A) and the LayerNorm affine (gamma into
weight columns, W@beta into bias) into the dense weights, so the device
runs pure GEMMs. LayerNorm itself is applied via a rank-2 correction
matmul (colsum(W) x (-mean*rstd) + bias x std) accumulated into each
projection PSUM plus a per-token rstd multiply at drain time, so GEMMs
consume raw bf16 activations and never wait on normalized tiles.

Attention: per key-block, both heads' score panels live in one 2-bank
PSUM tile so the two K=64 QK matmuls run concurrently in different PE
row-groups; Exp on Scalar, multiplicative 0/1 causal mask, AV matmuls
with an extra ones-column of V accumulating the softmax denominator.
The epilogue copies raw accumulators out to free PSUM quickly, then
reciprocal (fast-approx DVE), K=1 ones-matmul broadcast, and normalize
off the critical path. The cross-V GEMM is interleaved into
self-attention per head-pair to keep the PE array clock-gate warm.
"""

import os
import sys

sys.path.insert(0, "/opt/trn_rl_repo")

import numpy as np
import ml_dtypes
from contextlib import ExitStack

BF = ml_dtypes.bfloat16

C = 1024
H = 16
DH = 64
R = 16
SCALE = 1.0 / R
T = 1024
TQ = 512
NT = 8  # C / 128
EPS = 1e-5
NCORES = 8

# fp8 quantization scales (powers of two; inputs are ~N(0,1) with absmax
# comfortably under 240/SX, weights ~N(0,0.02) under 240/SW)
SX = 32.0   # x / feature activations
SR = 16.0   # residual -> rc casts
SW = 1024.0  # wq / wk / wcq / wck weights
C_K = 1.0 / (SW * SX)
C_Q = 1.0 / (SW * SR)
C_CK = 1.0 / (SW * SX)
F8 = ml_dtypes.float8_e4m3

_PROG = None


def _build_program():
    import math
    import concourse.bass as bass
    import concourse.tile as tile
    from concourse import mybir, bacc

    f32 = mybir.dt.float32
    bf16 = mybir.dt.bfloat16
    fp8 = mybir.dt.float8e4
    AF = mybir.ActivationFunctionType
    AL = mybir.AluOpType
    PM = mybir.MatmulPerfMode

    nc = bacc.Bacc("TRN2", target_bir_lowering=False, debug=False)

    def din(name, shape, dt=f32):
        return nc.dram_tensor(name, shape, dt, kind="ExternalInput").ap()

    xbT_d = din("xbT", [C, T], fp8)
    xqT_d = din("xqT", [C, TQ])
    fbT_d = din("fbT", [C, T], fp8)
    sqxT_d = din("sqxT", [C, T], bf16)
    band_d = din("band", [128, 128], bf16)

    w_d = {}
    for n in ["wv", "wsp", "wcv", "wcp"]:
        w_d[n] = din(n, [C, C], bf16)
    for n in ["wq", "wk", "wcq", "wck"]:
        w_d[n] = din(n, [C, C], fp8)
    w_d["wfc"] = din("wfc", [C, 4 * C], bf16)
    w_d["wpr"] = din("wpr", [4 * C, C], bf16)
    aux_d = {n: din(n, [2, C], bf16) for n in ["aux_q", "aux_k", "aux_v", "aux_cq"]}
    bias_d = {n: din(n, [C], f32) for n in ["bsp", "bck", "bcp", "bpr"]}
    bias_d["bfc"] = din("bfc", [4 * C], f32)
    bcvrow_d = din("bcv_row", [1, C], bf16)

    outT_d = nc.dram_tensor("outT", [C, TQ], f32, kind="ExternalOutput").ap()
    KDBG = os.environ.get("KDBG", "") == "1"
    dbg_d = {}
    if KDBG:
        for n, shp, dt in [("d_k2T", [C, T], bf16), ("d_rows2", [2, T], bf16),
                           ("d_rbf", [128, T], f32), ("d_qT", [C, TQ], bf16),
                           ("d_kT", [C, T], bf16), ("d_v", [C, 1040], bf16),
                           ("d_oT", [C, TQ], bf16), ("d_r1", [C, TQ], f32),
                           ("d_q2T", [C, TQ], bf16), ("d_o2T", [C, TQ], bf16),
                           ("d_r2", [C, TQ], f32), ("d_ln2", [C, TQ], bf16),
                           ("d_m", [2 * C, 1024], bf16)]:
            dbg_d[n] = nc.dram_tensor(n, shp, dt, kind="ExternalOutput").ap()

    with tile.TileContext(nc) as tc, ExitStack() as ctx:

        def pool(name, bufs, space=None):
            kw = dict(name=name, bufs=bufs)
            if space:
                kw["space"] = space
            return ctx.enter_context(tc.tile_pool(**kw))

        # SBUF pools
        actbig = pool("actbig", 16)   # [128,1024] bf16: xb(8)+fb(8) -> m_sb(16)
        kpool = pool("kpool", 8)      # [128,1024] bf16: kT
        k2pool = pool("k2pool", 8)    # [128,1024] bf16: k2T
        vpool = pool("vpool", 16)     # [128,1040] bf16: vt(8)+v2t(8)
        qpool = pool("qpool", 8)      # [128,512] bf16: qT -> q2T -> ln2
        opool = pool("opool", 8)      # [128,512] bf16: oT -> o2T
        rpool = pool("rpool", 8)      # [128,512] f32: residual (persist)
        rc8p = pool("rc8p", 4)        # [128,1024] fp8: rc1/rc2/rc3 pair tiles
        wpool = pool("wpool", 5)      # [128,1024] bf16 weight chunks
        w8pool = pool("w8pool", 5)    # [128,1024] fp8 weight chunks
        epool = pool("epool", 3)      # [128,1024] bf16: squares / exp(S)
        sbigT = pool("sbigT", 1)      # [128,1024] f32: rb bcast full-T
        sbigS = pool("sbigS", 2)      # [128,512] f32: small LN bcasts
        rows = pool("rows", 3)        # [1,1024] f32 stat rows (full T)
        srows = pool("srows", 3)      # [1,512] f32 stat rows (own)
        rbfp = pool("rbfp", 1)        # [1,<=1024] bf16 std rows
        rows2p = pool("rows2p", 1)    # [2,1024] bf16 correction rows
        rows2sp = pool("rows2sp", 2)  # [2,512] bf16 correction rows (own)
        rrp = pool("rrp", 2)          # [1,512] bf16 softmax denom rows
        rcolp = pool("rcolp", 1)      # [128,8] f32 rstd col-packed
        auxp = pool("auxp", 1)        # [2,1024] bf16 aux tensors
        smalls = pool("smalls", 1)    # [128,<=32] bias columns (per tag)
        onesp = pool("onesp", 1)
        bandp = pool("bandp", 1)
        bvp = pool("bvp", 1)

        # PSUM pools: 2x2 + 4x1 = 8 banks
        ps = pool("ps", 2, space="PSUM")   # [128,1024] f32: scores / stats / pr
        po = pool("po", 2, space="PSUM")   # [<=128,512] f32: attn out / proj
        pp = pool("pp", 2, space="PSUM")   # [128,512] f32: proj / denb

        # ---- constants ----
        ones_c16 = onesp.tile([128, 1], bf16, tag="oc16")
        nc.gpsimd.memset(ones_c16[:], 1.0)
        ones_r16 = onesp.tile([1, 128], bf16, tag="or16")
        nc.gpsimd.memset(ones_r16[:], 1.0)
        ones_r32 = onesp.tile([1, 128], f32, tag="or32")
        nc.gpsimd.memset(ones_r32[:], 1.0)
        eps_t = onesp.tile([1, 1], f32, tag="eps")
        nc.gpsimd.memset(eps_t[:], EPS)
        one_1x1 = onesp.tile([1, 1], bf16, tag="one11")
        nc.gpsimd.memset(one_1x1[:], 1.0)
        dum = onesp.tile([1, 8], f32, tag="dum")
        nc.gpsimd.memset(dum[:], 1.0)
        # ln(dequant-const) biases folded into the rstd Exp
        lnck_t = onesp.tile([1, 1], f32, tag="lnck")
        nc.gpsimd.memset(lnck_t[:], math.log(C_K))
        lncq_t = onesp.tile([1, 1], f32, tag="lncq")
        nc.gpsimd.memset(lncq_t[:], math.log(C_Q))
        lnsx_t = onesp.tile([1, 1], f32, tag="lnsx")
        nc.gpsimd.memset(lnsx_t[:], math.log(1.0 / SX))
        zero_t = onesp.tile([1, 1], f32, tag="zero")
        nc.gpsimd.memset(zero_t[:], 0.0)
        # prime the ln+exp activation table before anything depends on it
        nc.scalar.activation(dum[:], dum[:], AF.Ln, bias=eps_t[:])
        nc.scalar.activation(dum[:], dum[:], AF.Exp)

        # ---- activation loads (fp8 pair tiles: [128, (j=2, T)]) ----
        # xp first on the sync queue (weights queue behind it); resid + fp on
        # gpsimd so the scalar queue stays clear for attention exps.
        xp = [actbig.tile([128, 2 * T], fp8, tag="actbig", name=f"xp{i}")
              for i in range(4)]
        for kk in range(4):
            nc.sync.dma_start(
                xp[kk][:].rearrange("p (j t) -> p j t", j=2),
                xbT_d[2 * kk * 128:(2 * kk + 2) * 128, :].rearrange(
                    "(j p) t -> p j t", p=128))
        resid = []
        for k in range(NT):
            rt = rpool.tile([128, TQ], f32, tag="rpool")
            nc.gpsimd.dma_start(rt[:], xqT_d[k * 128:(k + 1) * 128, :])
            resid.append(rt)
        fp_ = [actbig.tile([128, 2 * T], fp8, tag="actbig", name=f"fp{i}")
               for i in range(4)]
        for kk in range(4):
            nc.gpsimd.dma_start(
                fp_[kk][:].rearrange("p (j t) -> p j t", j=2),
                fbT_d[2 * kk * 128:(2 * kk + 2) * 128, :].rearrange(
                    "(j p) t -> p j t", p=128))

        band2_t = bandp.tile([128, 128], bf16, tag="band")
        nc.gpsimd.dma_start(band2_t[:], band_d[:, :])

        def load_percol(name, n=NT):
            t = smalls.tile([128, n], f32, tag=name)
            nc.gpsimd.dma_start(t[:], bias_d[name].rearrange("(m p) -> p m", p=128))
            return t

        bias_t = {n: load_percol(n) for n in ["bsp", "bck", "bcp", "bpr"]}
        bias_t["bfc"] = load_percol("bfc", 32)
        bcv_t = bvp.tile([1, C], bf16, tag="bcv")
        nc.gpsimd.dma_start(bcv_t[:], bcvrow_d[:, :])
        aux_t = {}
        for n in ["aux_q", "aux_k", "aux_v", "aux_cq"]:
            a = auxp.tile([2, C], bf16, tag=n)
            nc.gpsimd.dma_start(a[:], aux_d[n][:, :])
            aux_t[n] = a

        def xbv(k, c0, c1):
            return xp[k // 2][:, (k % 2) * T + c0:(k % 2) * T + c1]

        def fbv(k, c0, c1):
            return fp_[k // 2][:, (k % 2) * T + c0:(k % 2) * T + c1]

        def xpair(kk, c0, c1):
            return xp[kk][:].rearrange("p (j t) -> p j t", j=2)[:, :, c0:c1]

        def fpair(kk, c0, c1):
            return fp_[kk][:].rearrange("p (j t) -> p j t", j=2)[:, :, c0:c1]

        # =============== helpers ===============
        def wload(wname, kk, mh, colbase=0, q=None):
            """[128,1024] tile holding k-blocks (2kk,2kk+1) of a 512-col half."""
            wt = wpool.tile([128, 1024], bf16, tag="wpool")
            src = w_d[wname][2 * kk * 128:(2 * kk + 2) * 128,
                             colbase + mh * 512:colbase + (mh + 1) * 512]
            (q or nc.sync).dma_start(
                wt[:].rearrange("p (j f) -> p j f", f=512),
                src.rearrange("(j p) f -> p j f", p=128))
            return wt

        def wload8(wname, kk, mh, q=None):
            """fp8 [128,1024] tile: k-pair (2kk,2kk+1) x one 512-col m-half."""
            wt = w8pool.tile([128, 1024], fp8, tag="w8")
            src = w_d[wname][2 * kk * 128:(2 * kk + 2) * 128,
                             mh * 512:(mh + 1) * 512]
            (q or nc.sync).dma_start(
                wt[:].rearrange("p (j f) -> p j f", f=512),
                src.rearrange("(j p) f -> p j f", p=128))
            return wt

        def dense_proj_dr(wname, pair_fn, Tn, drain, corr, wq_=None):
            """DoubleRow fp8 proj: out^T[mi] via 4 K=256 matmuls + bf16 corr.

            pair_fn(kk, c0, c1) -> [128, 2, c1-c0] fp8 rhs view.
            """
            pcnt = 0
            for mh in range(2):
                wts = [wload8(wname, kk, mh, q=wq_) for kk in range(4)]
                for ml in range(4):
                    mi = mh * 4 + ml
                    for h in range(Tn // 512):
                        pl, ptag = ((pp, "pp"), (po, "po"))[pcnt % 2]
                        pcnt += 1
                        pt = pl.tile([128, 512], f32, tag=ptag)
                        for kk in range(4):
                            lhsT = wts[kk][:].rearrange(
                                "p (j f) -> p j f", f=512)[:, :, ml * 128:(ml + 1) * 128]
                            nc.tensor.matmul(
                                pt[:], lhsT, pair_fn(kk, h * 512, (h + 1) * 512),
                                start=(kk == 0), stop=False,
                                perf_mode=PM.DoubleRow)
                        a_t, r2 = corr
                        nc.tensor.matmul(pt[:], a_t[:, mi * 128:(mi + 1) * 128],
                                         r2[:, h * 512:(h + 1) * 512],
                                         start=False, stop=True)
                        drain(mi, h, pt)

        def dense_proj(wname, rhs_tiles, Tn, drain, corr=None, pools=None, wq_=None):
            """out^T[mi] tiles via PE; optional K=2 LN-correction matmul.

            corr = (aux_tile, rows2_tile) accumulated as aux[:,mi]^T @ rows2.
            drain(mi, h, pt) consumes each [128,512] psum.
            """
            if pools is None:
                pools = ((pp, "pp"), (po, "po"))
            pcnt = 0
            for mh in range(2):
                wts = [wload(wname, kk, mh, q=wq_) for kk in range(4)]
                for ml in range(4):
                    mi = mh * 4 + ml
                    for h in range(Tn // 512):
                        sl = slice(h * 512, (h + 1) * 512)
                        pl, ptag = pools[pcnt % len(pools)]
                        pcnt += 1
                        pt = pl.tile([128, 512], f32, tag=ptag)
                        for k in range(NT):
                            kk, j = k // 2, k % 2
                            nc.tensor.matmul(
                                pt[:], wts[kk][:, j * 512 + ml * 128:j * 512 + (ml + 1) * 128],
                                rhs_tiles[k][:, sl], start=(k == 0),
                                stop=(k == NT - 1 and corr is None))
                        if corr is not None:
                            a_t, r2 = corr
                            nc.tensor.matmul(pt[:], a_t[:, mi * 128:(mi + 1) * 128],
                                             r2[:, sl], start=False, stop=True)
                        drain(mi, h, pt)

        def dense_projV(wname, lhs_fn, v_tiles, corr=None, bias_row=None,
                        rstd_col=None, drain_c=None, pools=None):
            """V natural [tok, dim]: fp8 activations stationary, bf16 w moving."""
            pcnt = 0
            if pools is None:
                pools = ((pp, "pp"), (po, "po"))
            for dh in range(2):
                sl = slice(dh * 512, (dh + 1) * 512)
                wts = [wload(wname, kk, dh) for kk in range(4)]
                for tt in range(NT):
                    pl, ptag = pools[pcnt % len(pools)]
                    pcnt += 1
                    pt = pl.tile([128, 512], f32, tag=ptag)
                    for k in range(NT):
                        kk, j = k // 2, k % 2
                        nc.tensor.matmul(
                            pt[:], lhs_fn(k, tt * 128, (tt + 1) * 128),
                            wts[kk][:, j * 512:(j + 1) * 512], start=(k == 0), stop=False)
                    if corr is not None:
                        a_t, r2 = corr
                        nc.tensor.matmul(pt[:], r2[:, tt * 128:(tt + 1) * 128],
                                         a_t[:, sl], start=False, stop=True)
                    else:
                        nc.tensor.matmul(pt[:], ones_r16[:], bias_row[:, sl],
                                         start=False, stop=True)
                    dest = v_tiles[tt][:, dh * 520:(dh + 1) * 520]
                    dest = dest.rearrange("p (h d) -> p h d", d=65)[:, :, 0:64]
                    if rstd_col is not None:
                        nc.vector.tensor_scalar_mul(dest, pt[:], rstd_col[:, tt:tt + 1])
                    else:
                        nc.vector.tensor_scalar_mul(dest, pt[:], drain_c)

        def ln_rows(x_fn, sq_tiles, Tn, rows_pool, rows2_tile, rb_tile,
                    rstd_col=None, negmr_out=None, mean_c=1.0 / C, sq_c=1.0 / C,
                    rb_bias=None, rcol_bias=None):
            """Stats over channel dim -> rows2 [2,Tn] (-m, std), rb bcast.

            x_fn(k, c0, c1) -> [128, c1-c0] view of (scaled) activations.
            mean_c/sq_c absorb the fp8 activation scale; rb_bias/rcol_bias are
            ln(dequant-const) folded into the Exp that produces rstd rows.
            """
            nh = Tn // 512
            mean_ps = ps.tile([1, Tn], f32, tag="ps")
            sq_ps = ps.tile([1, Tn], f32, tag="ps")
            for k in range(NT):
                for hh in range(nh):
                    sl = slice(hh * 512, (hh + 1) * 512)
                    nc.tensor.matmul(mean_ps[0:1, sl], ones_c16[:],
                                     x_fn(k, hh * 512, (hh + 1) * 512),
                                     start=(k == 0), stop=(k == NT - 1))
                    nc.tensor.matmul(sq_ps[0:1, sl], ones_c16[:], sq_tiles[k][:, sl],
                                     start=(k == 0), stop=(k == NT - 1))
            mean_row = rows_pool.tile([1, Tn], f32, tag="r")
            var_row = rows_pool.tile([1, Tn], f32, tag="r")
            rstd_row = rows_pool.tile([1, Tn], bf16, tag="r")
            nc.vector.tensor_scalar_mul(mean_row[:], mean_ps[:], mean_c)
            nc.vector.tensor_mul(var_row[:], mean_row[:], mean_row[:])
            nc.vector.scalar_tensor_tensor(var_row[:], sq_ps[:], sq_c, var_row[:],
                                           op0=AL.mult, op1=AL.subtract)
            # rstd*c = exp(-0.5*ln(var+eps) + ln c); std = exp(+0.5*ln(var+eps))
            nc.scalar.activation(var_row[:], var_row[:], AF.Ln, bias=eps_t[:])
            nc.scalar.activation(rstd_row[:], var_row[:], AF.Exp, scale=-0.5,
                                 bias=(zero_t[:] if rb_bias is None else rb_bias))
            if rows2_tile is not None:
                # rows2: row0 = -mean (bf16), row1 = std (bf16 via DMA)
                std_bf = rbfp.tile([1, Tn], bf16, tag="rbf")
                nc.scalar.activation(std_bf[:], var_row[:], AF.Exp, scale=0.5)
                nc.vector.tensor_scalar_mul(rows2_tile[0:1, :], mean_row[:], -1.0)
                nc.gpsimd.dma_start(rows2_tile[1:2, :], std_bf[:])
            # scaled-rstd broadcast [128,Tn] f32 via K=1 matmul
            for hh in range(nh):
                sl = slice(hh * 512, (hh + 1) * 512)
                bp = pp.tile([128, 512], f32, tag="pp")
                nc.tensor.matmul(bp[:], ones_r16[:], rstd_row[0:1, sl],
                                 start=True, stop=True)
                nc.vector.tensor_copy(rb_tile[:, sl], bp[:])
            if rstd_col is not None:
                # transpose per-token rstd*c_v into columns via K=1 MMs
                rstd_v = rows_pool.tile([1, Tn], bf16, tag="r")
                nc.scalar.activation(rstd_v[:], var_row[:], AF.Exp, scale=-0.5,
                                     bias=(zero_t[:] if rcol_bias is None else rcol_bias))
                rcps = pp.tile([128, NT], f32, tag="pp")
                for tt in range(NT):
                    nc.tensor.matmul(rcps[:, tt:tt + 1],
                                     rstd_v[0:1, tt * 128:(tt + 1) * 128],
                                     one_1x1[:], start=True, stop=True)
                nc.vector.tensor_copy(rstd_col[:], rcps[:])
            if negmr_out is not None:
                # broadcast of -mean*rstd for explicit normalize (rb_bias=0)
                nc.vector.scalar_tensor_tensor(var_row[:], mean_row[:], -1.0,
                                               rstd_row[:], op0=AL.mult, op1=AL.mult)
                bp = pp.tile([128, 512], f32, tag="pp")
                nc.tensor.matmul(bp[:], ones_r32[:], var_row[0:1, :],
                                 start=True, stop=True)
                nc.vector.tensor_copy(negmr_out[:], bp[:])

        def attention(q_tiles, k_tiles, v_tiles, o_tiles, filler=None):
            for mi in range(NT):
                opA = po.tile([65, 512], f32, tag="po", name=f"opA{mi}")
                opB = po.tile([65, 512], f32, tag="po", name=f"opB{mi}")
                hA, hB = 2 * mi, 2 * mi + 1
                for kj in range(8):
                    q0 = 64 * kj
                    st = ps.tile([128, 1024], f32, tag="ps")
                    # head A scores in cols [q0:512] (bank 0), head B in
                    # [512+q0:1024] (bank 1) -> row-tiled QKs run concurrently
                    nc.tensor.matmul(
                        st[:, q0:512],
                        k_tiles[mi][0:64, kj * 128:(kj + 1) * 128],
                        q_tiles[mi][0:64, q0:512], start=True, stop=True)
                    nc.tensor.matmul(
                        st[:, 512 + q0:1024],
                        k_tiles[mi][64:128, kj * 128:(kj + 1) * 128],
                        q_tiles[mi][64:128, q0:512], start=True, stop=True)
                    et = epool.tile([128, 1024], bf16, tag="e")
                    # one exp + one mask-mul covering both heads' strips
                    stv = st[:].rearrange("p (j t) -> p j t", j=2)[:, :, q0:512]
                    etv = et[:].rearrange("p (j t) -> p j t", j=2)[:, :, q0:512]
                    nc.scalar.activation(etv, stv, AF.Exp)
                    etm = et[:].rearrange("p (j t) -> p j t", j=2)[:, :, q0:q0 + 64]
                    b2v = band2_t[:].rearrange("p (j t) -> p j t", j=2)
                    nc.vector.tensor_mul(etm, etm, b2v)
                    nc.tensor.matmul(
                        opA[:] if kj == 0 else opA[:, q0:512],
                        v_tiles[kj][:, 65 * hA:65 * hA + 65],
                        et[:, q0:512], start=(kj == 0), stop=(kj == 7))
                    nc.tensor.matmul(
                        opB[:] if kj == 0 else opB[:, q0:512],
                        v_tiles[kj][:, 65 * hB:65 * hB + 65],
                        et[:, 512 + q0:1024], start=(kj == 0), stop=(kj == 7))
                # epilogue: copy raw accumulators + denom rows out (frees
                # PSUM fast), then recip/broadcast/normalize off-path
                osA = rrp.tile([64, 512], f32, tag="os")
                osB = rrp.tile([64, 512], f32, tag="os")
                rrA = rrp.tile([1, 512], f32, tag="rr")
                rrB = rrp.tile([1, 512], f32, tag="rr")
                nc.vector.tensor_copy(osA[:], opA[0:64, :])
                nc.vector.tensor_copy(osB[:], opB[0:64, :])
                nc.vector.tensor_copy(rrA[:], opA[64:65, :])
                nc.vector.tensor_copy(rrB[:], opB[64:65, :])
                nc.vector.reciprocal_approx_fast(rrA[:], rrA[:])
                nc.vector.reciprocal_approx_fast(rrB[:], rrB[:])
                rrAb = rrp.tile([1, 512], bf16, tag="rrb")
                rrBb = rrp.tile([1, 512], bf16, tag="rrb")
                nc.vector.tensor_copy(rrAb[:], rrA[:])
                nc.vector.tensor_copy(rrBb[:], rrB[:])
                # broadcast 1/den via K=1 matmul; multiply straight out of PSUM
                denbA = po.tile([64, 512], f32, tag="po", name=f"denbA{mi}")
                denbB = po.tile([64, 512], f32, tag="po", name=f"denbB{mi}")
                nc.tensor.matmul(denbA[:], ones_r16[0:1, 0:64], rrAb[:],
                                 start=True, stop=True)
                nc.tensor.matmul(denbB[:], ones_r16[0:1, 0:64], rrBb[:],
                                 start=True, stop=True)
                nc.vector.tensor_mul(o_tiles[mi][0:64, :], osA[:], denbA[:])
                nc.vector.tensor_mul(o_tiles[mi][64:128, :], osB[:], denbB[:])
                if filler is not None:
                    filler(2 * mi)
                    filler(2 * mi + 1)

        def make_cv_filler(wname, lhs_fn, v_tiles, bias_row):
            state = {}

            def chunk(c):
                dh, tt = c // 8, c % 8
                if tt == 0:
                    state["wts"] = [wload(wname, kk, dh) for kk in range(4)]
                wts = state["wts"]
                pt = pp.tile([128, 512], f32, tag="pp")
                for k in range(NT):
                    kk, j = k // 2, k % 2
                    nc.tensor.matmul(
                        pt[:], lhs_fn(k, tt * 128, (tt + 1) * 128),
                        wts[kk][:, j * 512:(j + 1) * 512], start=(k == 0), stop=False)
                nc.tensor.matmul(pt[:], ones_r16[:], bias_row[:, dh * 512:(dh + 1) * 512],
                                 start=False, stop=True)
                dest = v_tiles[tt][:, dh * 520:(dh + 1) * 520]
                dest = dest.rearrange("p (h d) -> p h d", d=65)[:, :, 0:64]
                nc.vector.tensor_scalar_mul(dest, pt[:], 1.0 / SX)

            return chunk

        def dump(name, tiles, rows=128):
            if not KDBG:
                return
            for i, t in enumerate(tiles):
                nc.gpsimd.dma_start(dbg_d[name][i * rows:(i + 1) * rows, :], t[:])

        # =============== phase 1: cross-K GEMM is deferred into the cross-
        # attention phase as PE filler (k2T[mi] produced 2 head-pairs ahead)
        k2T = [k2pool.tile([128, T], bf16, tag="k2", name=f"k2T{i}") for i in range(NT)]

        ck_state = {}

        def ck_chunk(mi):
            mh, ml = mi // 4, mi % 4
            if ml == 0:
                ck_state[mh] = [wload8("wck", kk, mh, q=nc.gpsimd) for kk in range(4)]
            wts = ck_state[mh]
            for h in range(2):
                sl = slice(h * 512, (h + 1) * 512)
                pt = pp.tile([128, 512], f32, tag="pp")
                for kk in range(4):
                    lhsT = wts[kk][:].rearrange(
                        "p (j f) -> p j f", f=512)[:, :, ml * 128:(ml + 1) * 128]
                    nc.tensor.matmul(pt[:], lhsT, fpair(kk, h * 512, (h + 1) * 512),
                                     start=(kk == 0), stop=(kk == 3),
                                     perf_mode=PM.DoubleRow)
                nc.vector.tensor_scalar(k2T[mi][:, sl], pt[:], C_CK,
                                        bias_t["bck"][:, mi:mi + 1],
                                        op0=AL.mult, op1=AL.add)

        def ck_filler(c):
            if c % 2 == 0:
                nxt = c // 2 + 2
                if nxt < NT:
                    ck_chunk(nxt)

        # =============== phase 2: LN1 stats over full x ===============
        # x^2 comes precomputed from the host (DVE fp8 reads are half-rate)
        sqx = []
        for k in range(NT):
            sq = epool.tile([128, T], bf16, tag="e")
            nc.scalar.dma_start(sq[:], sqxT_d[k * 128:(k + 1) * 128, :])
            sqx.append(sq)
        rows2 = rows2p.tile([2, T], bf16, tag="r2")
        rb_f = sbigT.tile([128, T], f32, tag="sbT")
        rcol = rcolp.tile([128, NT], f32, tag="rcol")
        ln_rows(xbv, sqx, T, rows, rows2, rb_f, rstd_col=rcol,
                mean_c=1.0 / (C * SX), sq_c=1.0 / C,
                rb_bias=lnck_t[:], rcol_bias=lnsx_t[:])
        dump("d_rows2", [rows2], rows=2)
        dump("d_rbf", [rb_f])

        # own-token stats from resid (rc = fp8 pair tiles of resid*SR)
        rc1 = [rc8p.tile([128, 2 * TQ], fp8, tag="rc8", name=f"rc1_{i}")
               for i in range(4)]

        def rc1v(k, c0, c1):
            return rc1[k // 2][:, (k % 2) * TQ + c0:(k % 2) * TQ + c1]

        def rc1pair(kk, c0, c1):
            return rc1[kk][:].rearrange("p (j t) -> p j t", j=2)[:, :, c0:c1]

        sq1 = []
        for k in range(NT):
            nc.vector.tensor_scalar_mul(rc1v(k, 0, TQ), resid[k][:], SR)
            sq = epool.tile([128, TQ], bf16, tag="e")
            nc.vector.tensor_mul(sq[:], resid[k][:], resid[k][:])
            sq1.append(sq)
        rows2o = rows2sp.tile([2, TQ], bf16, tag="r2s")
        rb_o = sbigS.tile([128, TQ], f32, tag="sbS")
        ln_rows(rc1v, sq1, TQ, srows, rows2o, rb_o,
                mean_c=1.0 / (C * SR), sq_c=1.0 / C,
                rb_bias=lncq_t[:])

        # =============== phase 3: k GEMM first (needs only full-x stats) =====
        kT = [kpool.tile([128, T], bf16, tag="k", name=f"kT{i}") for i in range(NT)]

        def k_drain(mi, h, pt):
            sl = slice(h * 512, (h + 1) * 512)
            nc.vector.tensor_mul(kT[mi][:, sl], pt[:], rb_f[:, sl])

        dense_proj_dr("wk", xpair, T, k_drain, corr=(aux_t["aux_k"], rows2), wq_=nc.scalar)

        vt = [vpool.tile([128, 1040], bf16, tag="v", name=f"vt{i}") for i in range(NT)]
        for tt in range(NT):
            nc.gpsimd.memset(vt[tt][:, 64:1040:65], 1.0)
        dense_projV("wv", xbv, vt, corr=(aux_t["aux_v"], rows2), rstd_col=rcol)
        qT = [qpool.tile([128, TQ], bf16, tag="q", name=f"qT{i}") for i in range(NT)]

        def q_drain(mi, h, pt):
            nc.vector.tensor_mul(qT[mi][:], pt[:], rb_o[:])

        dense_proj_dr("wq", rc1pair, TQ, q_drain, corr=(aux_t["aux_q"], rows2o), wq_=nc.scalar)

        dump("d_qT", qT)
        dump("d_kT", kT)
        dump("d_v", vt)

        # =============== phase 4: self attention + interleaved cross-V ======
        v2t = [vpool.tile([128, 1040], bf16, tag="v", name=f"v2t{i}") for i in range(NT)]
        for tt in range(NT):
            nc.gpsimd.memset(v2t[tt][:, 64:1040:65], 1.0)
        oT = [opool.tile([128, TQ], bf16, tag="o", name=f"oT{i}") for i in range(NT)]
        attention(qT, kT, vt, oT, filler=make_cv_filler("wcv", fbv, v2t, bcv_t))
        dump("d_oT", oT)

        # =============== phase 6: self proj + residual (rc2 cast fused) ======
        rc2 = [rc8p.tile([128, 2 * TQ], fp8, tag="rc8", name=f"rc2_{i}")
               for i in range(4)]

        def rc2v(k, c0, c1):
            return rc2[k // 2][:, (k % 2) * TQ + c0:(k % 2) * TQ + c1]

        def rc2pair(kk, c0, c1):
            return rc2[kk][:].rearrange("p (j t) -> p j t", j=2)[:, :, c0:c1]

        def sp_drain(mi, h, pt):
            nc.vector.scalar_tensor_tensor(resid[mi][:], pt[:], bias_t["bsp"][:, mi:mi + 1],
                                           resid[mi][:], op0=AL.add, op1=AL.add)
            nc.vector.tensor_scalar_mul(rc2v(mi, 0, TQ), resid[mi][:], SR)

        dense_proj("wsp", oT, TQ, sp_drain)
        dump("d_r1", resid)

        # =============== phase 7: LN1 on updated own tokens ===============
        sq2 = []
        for k in range(NT):
            sq = epool.tile([128, TQ], bf16, tag="e")
            nc.vector.tensor_mul(sq[:], resid[k][:], resid[k][:])
            sq2.append(sq)
        rows2o2 = rows2sp.tile([2, TQ], bf16, tag="r2s")
        rb_o2 = sbigS.tile([128, TQ], f32, tag="sbS")
        ln_rows(rc2v, sq2, TQ, srows, rows2o2, rb_o2,
                mean_c=1.0 / (C * SR), sq_c=1.0 / C,
                rb_bias=lncq_t[:])

        # =============== phase 8: cross q ===============
        q2T = [qpool.tile([128, TQ], bf16, tag="q", name=f"q2T{i}") for i in range(NT)]

        def q2_drain(mi, h, pt):
            nc.vector.tensor_mul(q2T[mi][:], pt[:], rb_o2[:])

        dense_proj_dr("wcq", rc2pair, TQ, q2_drain, corr=(aux_t["aux_cq"], rows2o2),
                      wq_=nc.gpsimd)
        dump("d_q2T", q2T)

        # =============== phase 9: cross attention ===============
        o2T = [opool.tile([128, TQ], bf16, tag="o", name=f"o2T{i}") for i in range(NT)]
        ck_chunk(0)
        ck_chunk(1)
        attention(q2T, k2T, v2t, o2T, filler=ck_filler)
        dump("d_k2T", k2T)
        dump("d_o2T", o2T)

        # =============== phase 10: cross proj + residual (rc3 cast fused) ====
        rc3 = [rc8p.tile([128, 2 * TQ], fp8, tag="rc8", name=f"rc3_{i}")
               for i in range(4)]

        def rc3v(k, c0, c1):
            return rc3[k // 2][:, (k % 2) * TQ + c0:(k % 2) * TQ + c1]

        def cp_drain(mi, h, pt):
            nc.vector.scalar_tensor_tensor(resid[mi][:], pt[:], bias_t["bcp"][:, mi:mi + 1],
                                           resid[mi][:], op0=AL.add, op1=AL.add)
            nc.vector.tensor_scalar_mul(rc3v(mi, 0, TQ), resid[mi][:], SR)

        dense_proj("wcp", o2T, TQ, cp_drain, wq_=nc.gpsimd)
        dump("d_r2", resid)

        # =============== phase 11: LN2 (explicit normalize) ===============
        sq3 = []
        for k in range(NT):
            sq = epool.tile([128, TQ], bf16, tag="e")
            nc.vector.tensor_mul(sq[:], resid[k][:], resid[k][:])
            sq3.append(sq)
        rb_o3 = sbigS.tile([128, TQ], f32, tag="sbS")
        nmr_o3 = sbigS.tile([128, TQ], f32, tag="sbS")
        ln_rows(rc3v, sq3, TQ, srows, None, rb_o3, negmr_out=nmr_o3,
                mean_c=1.0 / (C * SR), sq_c=1.0 / C)
        # prime the gelu table after the last ln/exp rows, before fc drains
        nc.scalar.activation(dum[:], dum[:], AF.Gelu_apprx_tanh)
        ln2 = [qpool.tile([128, TQ], bf16, tag="q", name=f"ln2_{i}") for i in range(NT)]
        for k in range(NT):
            nc.vector.tensor_mul(ln2[k][:], resid[k][:], rb_o3[:])
            nc.vector.tensor_add(ln2[k][:], ln2[k][:], nmr_o3[:])

        dump("d_ln2", ln2)

        # =============== phase 12: MLP fc ===============
        m_sb = [actbig.tile([128, 1024], bf16, tag="actbig", name=f"m{i}")
                for i in range(16)]
        pcnt = 0
        fpools = ((pp, "pp"), (po, "po"))
        for g in range(8):
            wts = [wload("wfc", kk, 0, colbase=g * 512) for kk in range(4)]
            for ml in range(4):
                mi = g * 4 + ml
                pl, ptag = fpools[pcnt % 2]
                pcnt += 1
                pt = pl.tile([128, 512], f32, tag=ptag)
                for k in range(NT):
                    kk, j = k // 2, k % 2
                    nc.tensor.matmul(pt[:], wts[kk][:, j * 512 + ml * 128:j * 512 + (ml + 1) * 128],
                                     ln2[k][:], start=(k == 0), stop=(k == NT - 1))
                nc.scalar.activation(m_sb[mi // 2][:, (mi % 2) * 512:(mi % 2 + 1) * 512],
                                     pt[:], AF.Gelu_apprx_tanh,
                                     bias=bias_t["bfc"][:, mi:mi + 1])

        dump("d_m", m_sb)

        # =============== phase 13: MLP proj, k-outer over all 8 psum banks ===
        psA = ps.tile([128, 1024], f32, tag="ps")
        psB = ps.tile([128, 1024], f32, tag="ps")
        poA = po.tile([128, 512], f32, tag="po")
        poB = po.tile([128, 512], f32, tag="po")
        ppA = pp.tile([128, 512], f32, tag="pp")
        ppB = pp.tile([128, 512], f32, tag="pp")
        prq = [psA[:, 0:512], psA[:, 512:1024], psB[:, 0:512], psB[:, 512:1024],
               poA[:], poB[:], ppA[:], ppB[:]]
        for k in range(32):
            wt = wpool.tile([128, 1024], bf16, tag="wpool")
            (nc.gpsimd if k % 2 == 0 else nc.sync).dma_start(
                wt[:], w_d["wpr"][k * 128:(k + 1) * 128, :])
            ms = m_sb[k // 2][:, (k % 2) * 512:(k % 2 + 1) * 512]
            for j in range(8):
                nc.tensor.matmul(prq[j], wt[:, j * 128:(j + 1) * 128], ms,
                                 start=(k == 0), stop=(k == 31))
        outqs = [nc.gpsimd, nc.scalar, nc.sync]
        for j in range(8):
            nc.vector.scalar_tensor_tensor(resid[j][:], prq[j], bias_t["bpr"][:, j:j + 1],
                                           resid[j][:], op0=AL.add, op1=AL.add)
            outqs[j % 3].dma_start(outT_d[j * 128:(j + 1) * 128, :], resid[j][:])

    nc.compile()
    return nc


def _get_program():
    global _PROG
    if _PROG is None:
        _PROG = _build_program()
    return _PROG


def _prep_shared(inputs):
    g = {}

    def bf(a):
        return np.ascontiguousarray(np.asarray(a, dtype=np.float32)).astype(BF)

    def f(a):
        return np.ascontiguousarray(np.asarray(a, dtype=np.float32))

    def q8(a, s):
        return np.clip(np.asarray(a, np.float32) * s, -240.0, 240.0).astype(F8)

    inv = 1.0 / np.sqrt(DH)
    g1, b1 = np.asarray(inputs["ln1_g"]), np.asarray(inputs["ln1_b"])
    g2, b2 = np.asarray(inputs["ln2_g"]), np.asarray(inputs["ln2_b"])

    def fold(w, b, a, lb, ln=None):
        W = np.asarray(w, np.float32) + SCALE * (np.asarray(lb, np.float32)
                                                 @ np.asarray(a, np.float32))
        beff = np.asarray(b, np.float32).copy()
        if ln is not None:
            gg, bb = ln
            beff = beff + W @ bb
            W = W * gg[None, :]
        return W, beff

    def aux_of(WT, beff):
        # rows: colsum (for -m*r term), bias (times std term)
        return bf(np.stack([WT.sum(axis=0), beff], axis=0))

    Wqkv, bqkv = fold(inputs["sa_qkv_w"], inputs["sa_qkv_b"],
                      inputs["sa_qkv_a"], inputs["sa_qkv_lb"], ln=(g1, b1))
    qw, kw, vw = (Wqkv[i * C:(i + 1) * C] for i in range(3))
    qb, kb, vb = (bqkv[i * C:(i + 1) * C] for i in range(3))
    g["wq"] = q8(qw.T * inv, SW)
    g["wk"] = q8(kw.T, SW)
    g["wv"] = bf(vw.T)
    g["aux_q"] = aux_of((qw.T * inv) * (SW * SR), qb * inv * (SW * SR))
    g["aux_k"] = aux_of(kw.T * (SW * SX), kb * (SW * SX))
    g["aux_v"] = aux_of(vw.T * SX, vb * SX)

    Wsp, bsp = fold(inputs["sa_proj_w"], inputs["sa_proj_b"],
                    inputs["sa_proj_a"], inputs["sa_proj_lb"])
    g["wsp"] = bf(Wsp.T)
    g["bsp"] = f(bsp)

    Wcq, bcq = fold(inputs["ca_q_w"], inputs["ca_q_b"],
                    inputs["ca_q_a"], inputs["ca_q_lb"], ln=(g1, b1))
    g["wcq"] = q8(Wcq.T * inv, SW)
    g["aux_cq"] = aux_of((Wcq.T * inv) * (SW * SR), bcq * inv * (SW * SR))

    Wckv, bckv = fold(inputs["ca_kv_w"], inputs["ca_kv_b"],
                      inputs["ca_kv_a"], inputs["ca_kv_lb"])
    g["wck"] = q8(Wckv[0:C].T, SW)
    g["wcv"] = bf(Wckv[C:2 * C].T)
    g["bck"] = f(bckv[0:C])
    g["bcv_row"] = bf(bckv[C:2 * C].reshape(1, C) * SX)

    Wcp, bcp = fold(inputs["ca_proj_w"], inputs["ca_proj_b"],
                    inputs["ca_proj_a"], inputs["ca_proj_lb"])
    g["wcp"] = bf(Wcp.T)
    g["bcp"] = f(bcp)

    Wfc = np.asarray(inputs["fc_w"], np.float32) * g2[None, :]
    bfc = np.asarray(inputs["fc_b"], np.float32) + np.asarray(inputs["fc_w"], np.float32) @ b2
    g["wfc"] = bf(Wfc.T)
    g["bfc"] = f(bfc)
    g["wpr"] = bf(np.asarray(inputs["pr_w"]).T)
    g["bpr"] = f(inputs["pr_b"])
    return g


def _make_in_maps(inputs):
    inputs = {k: np.asarray(v) for k, v in inputs.items()}
    x, feat = inputs["x"], inputs["feature"]
    B = x.shape[0]
    shared = _prep_shared(inputs)

    bands = []
    for p in range(2):
        jj = np.arange(128).reshape(128, 1)
        ii = np.arange(64).reshape(1, 64)
        b = np.where(jj <= 2 * ii + p, 1.0, 0.0).astype(np.float32).astype(BF)
        bands.append(np.concatenate([b, b], axis=1))  # duplicated for 2-head strip

    in_maps = []
    xTs = [np.ascontiguousarray(np.asarray(x[b]).T, dtype=np.float32) for b in range(B)]
    fTs = [np.ascontiguousarray(np.asarray(feat[b]).T, dtype=np.float32) for b in range(B)]
    x8s = [np.clip(t * SX, -240, 240).astype(F8) for t in xTs]
    f8s = [np.clip(t * SX, -240, 240).astype(F8) for t in fTs]
    sqs = [(t * t).astype(BF) for t in xTs]
    for core in range(NCORES):
        b, p = core // 2, core % 2
        m = dict(shared)
        m["xbT"] = x8s[b]
        m["xqT"] = np.ascontiguousarray(xTs[b][:, p::2])
        m["fbT"] = f8s[b]
        m["sqxT"] = sqs[b]
        m["band"] = bands[p]
        in_maps.append(m)
    return in_maps, B


def kernel(**inputs):
    from concourse.bass_utils import run_bass_kernel_spmd

    nc = _get_program()
    in_maps, B = _make_in_maps(inputs)
    res = run_bass_kernel_spmd(nc, in_maps, core_ids=list(range(NCORES)))
    out = np.zeros((B, T, C), np.float32)
    for core in range(NCORES):
        b, p = core // 2, core % 2
        out[b, p::2, :] = np.asarray(res.results[core]["outT"], dtype=np.float32).T
    return out



# revision 68
# speedup vs baseline: 1.0663x; 1.0663x over previous
"""Trainium2 Bass kernel for nn_Block_with_lora (dense transformer block).

Sharding: 8 cores = 4 batches x 2 token-parity shards (stride-2 over T).
Each core computes its 512 query tokens end-to-end (no collectives);
K/V projections over all 1024 tokens are computed per-core.

Host-side prep folds LoRA (W + s*B*A) and the LayerNorm affine (gamma into
weight columns, W@beta into bias) into the dense weights, so the device
runs pure GEMMs. LayerNorm itself is applied via a rank-2 correction
matmul (colsum(W) x (-mean*rstd) + bias x std) accumulated into each
projection PSUM plus a per-token rstd multiply at drain time, so GEMMs
consume raw bf16 activations and never wait on normalized tiles.

Attention: per key-block, both heads' score panels live in one 2-bank
PSUM tile so the two K=64 QK matmuls run concurrently in different PE
row-groups; Exp on Scalar, multiplicative 0/1 causal mask, AV matmuls
with an extra ones-column of V accumulating the softmax denominator.
The epilogue copies raw accumulators out to free PSUM quickly, then
reciprocal (fast-approx DVE), K=1 ones-matmul broadcast, and normalize
off the critical path. The cross-V GEMM is interleaved into
self-attention per head-pair to keep the PE array clock-gate warm.
"""

import os
import sys

sys.path.insert(0, "/opt/trn_rl_repo")

import numpy as np
import ml_dtypes
from contextlib import ExitStack

BF = ml_dtypes.bfloat16

C = 1024
H = 16
DH = 64
R = 16
SCALE = 1.0 / R
T = 1024
TQ = 512
NT = 8  # C / 128
EPS = 1e-5
NCORES = 8

# fp8 quantization scales (powers of two; inputs are ~N(0,1) with absmax
# comfortably under 240/SX, weights ~N(0,0.02) under 240/SW)
SX = 32.0   # x / feature activations
SR = 16.0   # residual -> rc casts
SW = 1024.0  # wq / wk / wcq / wck weights
C_K = 1.0 / (SW * SX)
C_Q = 1.0 / (SW * SR)
C_CK = 1.0 / (SW * SX)
F8 = ml_dtypes.float8_e4m3

_PROG = None


def _build_program():
    import math
    import concourse.bass as bass
    import concourse.tile as tile
    from concourse import mybir, bacc

    f32 = mybir.dt.float32
    bf16 = mybir.dt.bfloat16
    fp8 = mybir.dt.float8e4
    AF = mybir.ActivationFunctionType
    AL = mybir.AluOpType
    PM = mybir.MatmulPerfMode

    nc = bacc.Bacc("TRN2", target_bir_lowering=False, debug=False)

    def din(name, shape, dt=f32):
        return nc.dram_tensor(name, shape, dt, kind="ExternalInput").ap()

    xbT_d = din("xbT", [C, T], fp8)
    xqT_d = din("xqT", [C, TQ])
    fbT_d = din("fbT", [C, T], fp8)
    sqxT_d = din("sqxT", [C, T], bf16)
    band_d = din("band", [128, 128], bf16)

    w_d = {}
    for n in ["wv", "wsp", "wcv", "wcp"]:
        w_d[n] = din(n, [C, C], bf16)
    for n in ["wq", "wk", "wcq", "wck"]:
        w_d[n] = din(n, [C, C], fp8)
    w_d["wfc"] = din("wfc", [C, 4 * C], bf16)
    w_d["wpr"] = din("wpr", [4 * C, C], bf16)
    aux_d = {n: din(n, [2, C], bf16) for n in ["aux_q", "aux_k", "aux_v", "aux_cq"]}
    bias_d = {n: din(n, [C], f32) for n in ["bsp", "bck", "bcp", "bpr"]}
    bias_d["bfc"] = din("bfc", [4 * C], f32)
    bcvrow_d = din("bcv_row", [1, C], bf16)

    outT_d = nc.dram_tensor("outT", [C, TQ], f32, kind="ExternalOutput").ap()
    KDBG = os.environ.get("KDBG", "") == "1"
    dbg_d = {}
    if KDBG:
        for n, shp, dt in [("d_k2T", [C, T], bf16), ("d_rows2", [2, T], bf16),
                           ("d_rbf", [128, T], f32), ("d_qT", [C, TQ], bf16),
                           ("d_kT", [C, T], bf16), ("d_v", [C, 1040], bf16),
                           ("d_oT", [C, TQ], bf16), ("d_r1", [C, TQ], f32),
                           ("d_q2T", [C, TQ], bf16), ("d_o2T", [C, TQ], bf16),
                           ("d_r2", [C, TQ], f32), ("d_ln2", [C, TQ], bf16),
                           ("d_m", [2 * C, 1024], bf16)]:
            dbg_d[n] = nc.dram_tensor(n, shp, dt, kind="ExternalOutput").ap()

    with tile.TileContext(nc) as tc, ExitStack() as ctx:

        def pool(name, bufs, space=None):
            kw = dict(name=name, bufs=bufs)
            if space:
                kw["space"] = space
            return ctx.enter_context(tc.tile_pool(**kw))

        # SBUF pools
        actbig = pool("actbig", 16)   # [128,1024] bf16: xb(8)+fb(8) -> m_sb(16)
        kpool = pool("kpool", 8)      # [128,1024] bf16: kT
        k2pool = pool("k2pool", 8)    # [128,1024] bf16: k2T
        vpool = pool("vpool", 16)     # [128,1040] bf16: vt(8)+v2t(8)
        qpool = pool("qpool", 8)      # [128,512] bf16: qT -> q2T -> ln2
        opool = pool("opool", 8)      # [128,512] bf16: oT -> o2T
        rpool = pool("rpool", 8)      # [128,512] f32: residual (persist)
        rc8p = pool("rc8p", 4)        # [128,1024] fp8: rc1/rc2/rc3 pair tiles
        wpool = pool("wpool", 5)      # [128,1024] bf16 weight chunks
        w8pool = pool("w8pool", 5)    # [128,1024] fp8 weight chunks
        epool = pool("epool", 3)      # [128,1024] bf16: squares / exp(S)
        sbigT = pool("sbigT", 1)      # [128,1024] f32: rb bcast full-T
        sbigS = pool("sbigS", 2)      # [128,512] f32: small LN bcasts
        rows = pool("rows", 3)        # [1,1024] f32 stat rows (full T)
        srows = pool("srows", 3)      # [1,512] f32 stat rows (own)
        rbfp = pool("rbfp", 1)        # [1,<=1024] bf16 std rows
        rows2p = pool("rows2p", 1)    # [2,1024] bf16 correction rows
        rows2sp = pool("rows2sp", 2)  # [2,512] bf16 correction rows (own)
        rrp = pool("rrp", 2)          # [1,512] bf16 softmax denom rows
        rcolp = pool("rcolp", 1)      # [128,8] f32 rstd col-packed
        auxp = pool("auxp", 1)        # [2,1024] bf16 aux tensors
        smalls = pool("smalls", 1)    # [128,<=32] bias columns (per tag)
        onesp = pool("onesp", 1)
        bandp = pool("bandp", 1)
        bvp = pool("bvp", 1)

        # PSUM pools: 2x2 + 4x1 = 8 banks
        ps = pool("ps", 2, space="PSUM")   # [128,1024] f32: scores / stats / pr
        po = pool("po", 2, space="PSUM")   # [<=128,512] f32: attn out / proj
        pp = pool("pp", 2, space="PSUM")   # [128,512] f32: proj / denb

        # ---- constants ----
        ones_c16 = onesp.tile([128, 1], bf16, tag="oc16")
        nc.gpsimd.memset(ones_c16[:], 1.0)
        ones_r16 = onesp.tile([1, 128], bf16, tag="or16")
        nc.gpsimd.memset(ones_r16[:], 1.0)
        ones_r32 = onesp.tile([1, 128], f32, tag="or32")
        nc.gpsimd.memset(ones_r32[:], 1.0)
        eps_t = onesp.tile([1, 1], f32, tag="eps")
        nc.gpsimd.memset(eps_t[:], EPS)
        one_1x1 = onesp.tile([1, 1], bf16, tag="one11")
        nc.gpsimd.memset(one_1x1[:], 1.0)
        dum = onesp.tile([1, 8], f32, tag="dum")
        nc.gpsimd.memset(dum[:], 1.0)
        # ln(dequant-const) biases folded into the rstd Exp
        lnck_t = onesp.tile([1, 1], f32, tag="lnck")
        nc.gpsimd.memset(lnck_t[:], math.log(C_K))
        lncq_t = onesp.tile([1, 1], f32, tag="lncq")
        nc.gpsimd.memset(lncq_t[:], math.log(C_Q))
        lnsx_t = onesp.tile([1, 1], f32, tag="lnsx")
        nc.gpsimd.memset(lnsx_t[:], math.log(1.0 / SX))
        zero_t = onesp.tile([1, 1], f32, tag="zero")
        nc.gpsimd.memset(zero_t[:], 0.0)
        # prime the ln+exp activation table before anything depends on it
        nc.scalar.activation(dum[:], dum[:], AF.Ln, bias=eps_t[:])
        nc.scalar.activation(dum[:], dum[:], AF.Exp)

        # ---- activation loads (fp8 pair tiles: [128, (j=2, T)]) ----
        # xp first on the sync queue (weights queue behind it); resid + fp on
        # gpsimd so the scalar queue stays clear for attention exps.
        xp = [actbig.tile([128, 2 * T], fp8, tag="actbig", name=f"xp{i}")
              for i in range(4)]
        for kk in range(4):
            nc.sync.dma_start(
                xp[kk][:].rearrange("p (j t) -> p j t", j=2),
                xbT_d[2 * kk * 128:(2 * kk + 2) * 128, :].rearrange(
                    "(j p) t -> p j t", p=128))
        resid = []
        for k in range(NT):
            rt = rpool.tile([128, TQ], f32, tag="rpool")
            nc.gpsimd.dma_start(rt[:], xqT_d[k * 128:(k + 1) * 128, :])
            resid.append(rt)
        fp_ = [actbig.tile([128, 2 * T], fp8, tag="actbig", name=f"fp{i}")
               for i in range(4)]
        for kk in range(4):
            nc.gpsimd.dma_start(
                fp_[kk][:].rearrange("p (j t) -> p j t", j=2),
                fbT_d[2 * kk * 128:(2 * kk + 2) * 128, :].rearrange(
                    "(j p) t -> p j t", p=128))

        band2_t = bandp.tile([128, 128], bf16, tag="band")
        nc.gpsimd.dma_start(band2_t[:], band_d[:, :])

        def load_percol(name, n=NT):
            t = smalls.tile([128, n], f32, tag=name)
            nc.gpsimd.dma_start(t[:], bias_d[name].rearrange("(m p) -> p m", p=128))
            return t

        bias_t = {n: load_percol(n) for n in ["bsp", "bck", "bcp", "bpr"]}
        bias_t["bfc"] = load_percol("bfc", 32)
        bcv_t = bvp.tile([1, C], bf16, tag="bcv")
        nc.gpsimd.dma_start(bcv_t[:], bcvrow_d[:, :])
        aux_t = {}
        for n in ["aux_q", "aux_k", "aux_v", "aux_cq"]:
            a = auxp.tile([2, C], bf16, tag=n)
            nc.gpsimd.dma_start(a[:], aux_d[n][:, :])
            aux_t[n] = a

        def xbv(k, c0, c1):
            return xp[k // 2][:, (k % 2) * T + c0:(k % 2) * T + c1]

        def fbv(k, c0, c1):
            return fp_[k // 2][:, (k % 2) * T + c0:(k % 2) * T + c1]

        def xpair(kk, c0, c1):
            return xp[kk][:].rearrange("p (j t) -> p j t", j=2)[:, :, c0:c1]

        def fpair(kk, c0, c1):
            return fp_[kk][:].rearrange("p (j t) -> p j t", j=2)[:, :, c0:c1]

        # =============== helpers ===============
        def wload(wname, kk, mh, colbase=0, q=None):
            """[128,1024] tile holding k-blocks (2kk,2kk+1) of a 512-col half."""
            wt = wpool.tile([128, 1024], bf16, tag="wpool")
            src = w_d[wname][2 * kk * 128:(2 * kk + 2) * 128,
                             colbase + mh * 512:colbase + (mh + 1) * 512]
            (q or nc.sync).dma_start(
                wt[:].rearrange("p (j f) -> p j f", f=512),
                src.rearrange("(j p) f -> p j f", p=128))
            return wt

        def wload8(wname, kk, mh, q=None):
            """fp8 [128,1024] tile: k-pair (2kk,2kk+1) x one 512-col m-half."""
            wt = w8pool.tile([128, 1024], fp8, tag="w8")
            src = w_d[wname][2 * kk * 128:(2 * kk + 2) * 128,
                             mh * 512:(mh + 1) * 512]
            (q or nc.sync).dma_start(
                wt[:].rearrange("p (j f) -> p j f", f=512),
                src.rearrange("(j p) f -> p j f", p=128))
            return wt

        def dense_proj_dr(wname, pair_fn, Tn, drain, corr, wq_=None):
            """DoubleRow fp8 proj: out^T[mi] via 4 K=256 matmuls + bf16 corr.

            pair_fn(kk, c0, c1) -> [128, 2, c1-c0] fp8 rhs view.
            """
            pcnt = 0
            for mh in range(2):
                wts = [wload8(wname, kk, mh, q=wq_) for kk in range(4)]
                for ml in range(4):
                    mi = mh * 4 + ml
                    for h in range(Tn // 512):
                        pl, ptag = ((pp, "pp"), (po, "po"))[pcnt % 2]
                        pcnt += 1
                        pt = pl.tile([128, 512], f32, tag=ptag)
                        for kk in range(4):
                            lhsT = wts[kk][:].rearrange(
                                "p (j f) -> p j f", f=512)[:, :, ml * 128:(ml + 1) * 128]
                            nc.tensor.matmul(
                                pt[:], lhsT, pair_fn(kk, h * 512, (h + 1) * 512),
                                start=(kk == 0), stop=False,
                                perf_mode=PM.DoubleRow)
                        a_t, r2 = corr
                        nc.tensor.matmul(pt[:], a_t[:, mi * 128:(mi + 1) * 128],
                                         r2[:, h * 512:(h + 1) * 512],
                                         start=False, stop=True)
                        drain(mi, h, pt)

        def dense_proj(wname, rhs_tiles, Tn, drain, corr=None, pools=None, wq_=None):
            """out^T[mi] tiles via PE; optional K=2 LN-correction matmul.

            corr = (aux_tile, rows2_tile) accumulated as aux[:,mi]^T @ rows2.
            drain(mi, h, pt) consumes each [128,512] psum.
            """
            if pools is None:
                pools = ((pp, "pp"), (po, "po"))
            pcnt = 0
            for mh in range(2):
                wts = [wload(wname, kk, mh, q=wq_) for kk in range(4)]
                for ml in range(4):
                    mi = mh * 4 + ml
                    for h in range(Tn // 512):
                        sl = slice(h * 512, (h + 1) * 512)
                        pl, ptag = pools[pcnt % len(pools)]
                        pcnt += 1
                        pt = pl.tile([128, 512], f32, tag=ptag)
                        for k in range(NT):
                            kk, j = k // 2, k % 2
                            nc.tensor.matmul(
                                pt[:], wts[kk][:, j * 512 + ml * 128:j * 512 + (ml + 1) * 128],
                                rhs_tiles[k][:, sl], start=(k == 0),
                                stop=(k == NT - 1 and corr is None))
                        if corr is not None:
                            a_t, r2 = corr
                            nc.tensor.matmul(pt[:], a_t[:, mi * 128:(mi + 1) * 128],
                                             r2[:, sl], start=False, stop=True)
                        drain(mi, h, pt)

        def dense_projV(wname, lhs_fn, v_tiles, corr=None, bias_row=None,
                        rstd_col=None, drain_c=None, pools=None):
            """V natural [tok, dim]: fp8 activations stationary, bf16 w moving."""
            pcnt = 0
            if pools is None:
                pools = ((pp, "pp"), (po, "po"))
            for dh in range(2):
                sl = slice(dh * 512, (dh + 1) * 512)
                wts = [wload(wname, kk, dh) for kk in range(4)]
                for tt in range(NT):
                    pl, ptag = pools[pcnt % len(pools)]
                    pcnt += 1
                    pt = pl.tile([128, 512], f32, tag=ptag)
                    for k in range(NT):
                        kk, j = k // 2, k % 2
                        nc.tensor.matmul(
                            pt[:], lhs_fn(k, tt * 128, (tt + 1) * 128),
                            wts[kk][:, j * 512:(j + 1) * 512], start=(k == 0), stop=False)
                    if corr is not None:
                        a_t, r2 = corr
                        nc.tensor.matmul(pt[:], r2[:, tt * 128:(tt + 1) * 128],
                                         a_t[:, sl], start=False, stop=True)
                    else:
                        nc.tensor.matmul(pt[:], ones_r16[:], bias_row[:, sl],
                                         start=False, stop=True)
                    dest = v_tiles[tt][:, dh * 520:(dh + 1) * 520]
                    dest = dest.rearrange("p (h d) -> p h d", d=65)[:, :, 0:64]
                    if rstd_col is not None:
                        nc.vector.tensor_scalar_mul(dest, pt[:], rstd_col[:, tt:tt + 1])
                    else:
                        nc.vector.tensor_scalar_mul(dest, pt[:], drain_c)

        def ln_rows(x_fn, sq_tiles, Tn, rows_pool, rows2_tile, rb_tile,
                    rstd_col=None, negmr_out=None, mean_c=1.0 / C, sq_c=1.0 / C,
                    rb_bias=None, rcol_bias=None):
            """Stats over channel dim -> rows2 [2,Tn] (-m, std), rb bcast.

            x_fn(k, c0, c1) -> [128, c1-c0] view of (scaled) activations.
            mean_c/sq_c absorb the fp8 activation scale; rb_bias/rcol_bias are
            ln(dequant-const) folded into the Exp that produces rstd rows.
            """
            nh = Tn // 512
            mean_ps = ps.tile([1, Tn], f32, tag="ps")
            sq_ps = ps.tile([1, Tn], f32, tag="ps")
            for k in range(NT):
                for hh in range(nh):
                    sl = slice(hh * 512, (hh + 1) * 512)
                    nc.tensor.matmul(mean_ps[0:1, sl], ones_c16[:],
                                     x_fn(k, hh * 512, (hh + 1) * 512),
                                     start=(k == 0), stop=(k == NT - 1))
                    nc.tensor.matmul(sq_ps[0:1, sl], ones_c16[:], sq_tiles[k][:, sl],
                                     start=(k == 0), stop=(k == NT - 1))
            mean_row = rows_pool.tile([1, Tn], f32, tag="r")
            var_row = rows_pool.tile([1, Tn], f32, tag="r")
            rstd_row = rows_pool.tile([1, Tn], bf16, tag="r")
            nc.vector.tensor_scalar_mul(mean_row[:], mean_ps[:], mean_c)
            nc.vector.tensor_mul(var_row[:], mean_row[:], mean_row[:])
            nc.vector.scalar_tensor_tensor(var_row[:], sq_ps[:], sq_c, var_row[:],
                                           op0=AL.mult, op1=AL.subtract)
            # rstd*c = exp(-0.5*ln(var+eps) + ln c); std = exp(+0.5*ln(var+eps))
            nc.scalar.activation(var_row[:], var_row[:], AF.Ln, bias=eps_t[:])
            nc.scalar.activation(rstd_row[:], var_row[:], AF.Exp, scale=-0.5,
                                 bias=(zero_t[:] if rb_bias is None else rb_bias))
            if rows2_tile is not None:
                # rows2: row0 = -mean (bf16), row1 = std (bf16 via DMA)
                std_bf = rbfp.tile([1, Tn], bf16, tag="rbf")
                nc.scalar.activation(std_bf[:], var_row[:], AF.Exp, scale=0.5)
                nc.vector.tensor_scalar_mul(rows2_tile[0:1, :], mean_row[:], -1.0)
                nc.gpsimd.dma_start(rows2_tile[1:2, :], std_bf[:])
            # scaled-rstd broadcast [128,Tn] f32 via K=1 matmul
            for hh in range(nh):
                sl = slice(hh * 512, (hh + 1) * 512)
                bp = pp.tile([128, 512], f32, tag="pp")
                nc.tensor.matmul(bp[:], ones_r16[:], rstd_row[0:1, sl],
                                 start=True, stop=True)
                nc.vector.tensor_copy(rb_tile[:, sl], bp[:])
            if rstd_col is not None:
                # transpose per-token rstd*c_v into columns via K=1 MMs
                rstd_v = rows_pool.tile([1, Tn], bf16, tag="r")
                nc.scalar.activation(rstd_v[:], var_row[:], AF.Exp, scale=-0.5,
                                     bias=(zero_t[:] if rcol_bias is None else rcol_bias))
                rcps = pp.tile([128, NT], f32, tag="pp")
                for tt in range(NT):
                    nc.tensor.matmul(rcps[:, tt:tt + 1],
                                     rstd_v[0:1, tt * 128:(tt + 1) * 128],
                                     one_1x1[:], start=True, stop=True)
                nc.vector.tensor_copy(rstd_col[:], rcps[:])
            if negmr_out is not None:
                # broadcast of -mean*rstd for explicit normalize (rb_bias=0)
                nc.vector.scalar_tensor_tensor(var_row[:], mean_row[:], -1.0,
                                               rstd_row[:], op0=AL.mult, op1=AL.mult)
                bp = pp.tile([128, 512], f32, tag="pp")
                nc.tensor.matmul(bp[:], ones_r32[:], var_row[0:1, :],
                                 start=True, stop=True)
                nc.vector.tensor_copy(negmr_out[:], bp[:])

        def attention(q_tiles, k_tiles, v_tiles, o_tiles, filler=None):
            for mi in range(NT):
                opA = po.tile([65, 512], f32, tag="po", name=f"opA{mi}")
                opB = po.tile([65, 512], f32, tag="po", name=f"opB{mi}")
                hA, hB = 2 * mi, 2 * mi + 1
                for kj in range(8):
                    q0 = 64 * kj
                    st = ps.tile([128, 1024], f32, tag="ps")
                    # head A scores in cols [q0:512] (bank 0), head B in
                    # [512+q0:1024] (bank 1) -> row-tiled QKs run concurrently
                    nc.tensor.matmul(
                        st[:, q0:512],
                        k_tiles[mi][0:64, kj * 128:(kj + 1) * 128],
                        q_tiles[mi][0:64, q0:512], start=True, stop=True)
                    nc.tensor.matmul(
                        st[:, 512 + q0:1024],
                        k_tiles[mi][64:128, kj * 128:(kj + 1) * 128],
                        q_tiles[mi][64:128, q0:512], start=True, stop=True)
                    et = epool.tile([128, 1024], bf16, tag="e")
                    # one exp + one mask-mul covering both heads' strips
                    stv = st[:].rearrange("p (j t) -> p j t", j=2)[:, :, q0:512]
                    etv = et[:].rearrange("p (j t) -> p j t", j=2)[:, :, q0:512]
                    nc.scalar.activation(etv, stv, AF.Exp)
                    etm = et[:].rearrange("p (j t) -> p j t", j=2)[:, :, q0:q0 + 64]
                    b2v = band2_t[:].rearrange("p (j t) -> p j t", j=2)
                    nc.vector.tensor_mul(etm, etm, b2v)
                    nc.tensor.matmul(
                        opA[:] if kj == 0 else opA[:, q0:512],
                        v_tiles[kj][:, 65 * hA:65 * hA + 65],
                        et[:, q0:512], start=(kj == 0), stop=(kj == 7))
                    nc.tensor.matmul(
                        opB[:] if kj == 0 else opB[:, q0:512],
                        v_tiles[kj][:, 65 * hB:65 * hB + 65],
                        et[:, 512 + q0:1024], start=(kj == 0), stop=(kj == 7))
                # epilogue: copy raw accumulators + denom rows out (frees
                # PSUM fast), then recip/broadcast/normalize off-path
                osA = rrp.tile([64, 512], f32, tag="os")
                osB = rrp.tile([64, 512], f32, tag="os")
                rrA = rrp.tile([1, 512], f32, tag="rr")
                rrB = rrp.tile([1, 512], f32, tag="rr")
                nc.vector.tensor_copy(osA[:], opA[0:64, :])
                nc.vector.tensor_copy(osB[:], opB[0:64, :])
                nc.vector.tensor_copy(rrA[:], opA[64:65, :])
                nc.vector.tensor_copy(rrB[:], opB[64:65, :])
                nc.vector.reciprocal_approx_fast(rrA[:], rrA[:])
                nc.vector.reciprocal_approx_fast(rrB[:], rrB[:])
                rrAb = rrp.tile([1, 512], bf16, tag="rrb")
                rrBb = rrp.tile([1, 512], bf16, tag="rrb")
                nc.vector.tensor_copy(rrAb[:], rrA[:])
                nc.vector.tensor_copy(rrBb[:], rrB[:])
                # broadcast 1/den via K=1 matmul; multiply straight out of PSUM
                denbA = po.tile([64, 512], f32, tag="po", name=f"denbA{mi}")
                denbB = po.tile([64, 512], f32, tag="po", name=f"denbB{mi}")
                nc.tensor.matmul(denbA[:], ones_r16[0:1, 0:64], rrAb[:],
                                 start=True, stop=True)
                nc.tensor.matmul(denbB[:], ones_r16[0:1, 0:64], rrBb[:],
                                 start=True, stop=True)
                nc.vector.tensor_mul(o_tiles[mi][0:64, :], osA[:], denbA[:])
                nc.vector.tensor_mul(o_tiles[mi][64:128, :], osB[:], denbB[:])
                if filler is not None:
                    filler(2 * mi)
                    filler(2 * mi + 1)

        def make_cv_filler(wname, lhs_fn, v_tiles, bias_row):
            state = {}

            def chunk(c):
                dh, tt = c // 8, c % 8
                if tt == 0:
                    state["wts"] = [wload(wname, kk, dh) for kk in range(4)]
                wts = state["wts"]
                pt = pp.tile([128, 512], f32, tag="pp")
                for k in range(NT):
                    kk, j = k // 2, k % 2
                    nc.tensor.matmul(
                        pt[:], lhs_fn(k, tt * 128, (tt + 1) * 128),
                        wts[kk][:, j * 512:(j + 1) * 512], start=(k == 0), stop=False)
                nc.tensor.matmul(pt[:], ones_r16[:], bias_row[:, dh * 512:(dh + 1) * 512],
                                 start=False, stop=True)
                dest = v_tiles[tt][:, dh * 520:(dh + 1) * 520]
                dest = dest.rearrange("p (h d) -> p h d", d=65)[:, :, 0:64]
                nc.vector.tensor_scalar_mul(dest, pt[:], 1.0 / SX)

            return chunk

        def dump(name, tiles, rows=128):
            if not KDBG:
                return
            for i, t in enumerate(tiles):
                nc.gpsimd.dma_start(dbg_d[name][i * rows:(i + 1) * rows, :], t[:])

        # =============== phase 1: cross-K GEMM is deferred into the cross-
        # attention phase as PE filler (k2T[mi] produced 2 head-pairs ahead)
        k2T = [k2pool.tile([128, T], bf16, tag="k2", name=f"k2T{i}") for i in range(NT)]

        ck_state = {}

        def ck_chunk(mi):
            mh, ml = mi // 4, mi % 4
            if ml == 0:
                ck_state[mh] = [wload8("wck", kk, mh, q=nc.gpsimd) for kk in range(4)]
            wts = ck_state[mh]
            for h in range(2):
                sl = slice(h * 512, (h + 1) * 512)
                pt = pp.tile([128, 512], f32, tag="pp")
                for kk in range(4):
                    lhsT = wts[kk][:].rearrange(
                        "p (j f) -> p j f", f=512)[:, :, ml * 128:(ml + 1) * 128]
                    nc.tensor.matmul(pt[:], lhsT, fpair(kk, h * 512, (h + 1) * 512),
                                     start=(kk == 0), stop=(kk == 3),
                                     perf_mode=PM.DoubleRow)
                nc.vector.tensor_scalar(k2T[mi][:, sl], pt[:], C_CK,
                                        bias_t["bck"][:, mi:mi + 1],
                                        op0=AL.mult, op1=AL.add)

        def ck_filler(c):
            if c % 2 == 0:
                nxt = c // 2 + 2
                if nxt < NT:
                    ck_chunk(nxt)

        # =============== phase 2: LN1 stats over full x ===============
        # x^2 comes precomputed from the host (DVE fp8 reads are half-rate)
        sqx = []
        for k in range(NT):
            sq = epool.tile([128, T], bf16, tag="e")
            nc.scalar.dma_start(sq[:], sqxT_d[k * 128:(k + 1) * 128, :])
            sqx.append(sq)
        rows2 = rows2p.tile([2, T], bf16, tag="r2")
        rb_f = sbigT.tile([128, T], f32, tag="sbT")
        rcol = rcolp.tile([128, NT], f32, tag="rcol")
        ln_rows(xbv, sqx, T, rows, rows2, rb_f, rstd_col=rcol,
                mean_c=1.0 / (C * SX), sq_c=1.0 / C,
                rb_bias=lnck_t[:], rcol_bias=lnsx_t[:])
        dump("d_rows2", [rows2], rows=2)
        dump("d_rbf", [rb_f])

        # =============== phase 3: k GEMM first (needs only full-x stats) =====
        kT = [kpool.tile([128, T], bf16, tag="k", name=f"kT{i}") for i in range(NT)]

        def k_drain(mi, h, pt):
            sl = slice(h * 512, (h + 1) * 512)
            nc.vector.tensor_mul(kT[mi][:, sl], pt[:], rb_f[:, sl])

        dense_proj_dr("wk", xpair, T, k_drain, corr=(aux_t["aux_k"], rows2), wq_=nc.scalar)

        # own-token stats from resid (rc = fp8 pair tiles of resid*SR)
        rc1 = [rc8p.tile([128, 2 * TQ], fp8, tag="rc8", name=f"rc1_{i}")
               for i in range(4)]

        def rc1v(k, c0, c1):
            return rc1[k // 2][:, (k % 2) * TQ + c0:(k % 2) * TQ + c1]

        def rc1pair(kk, c0, c1):
            return rc1[kk][:].rearrange("p (j t) -> p j t", j=2)[:, :, c0:c1]

        sq1 = []
        for k in range(NT):
            nc.vector.tensor_scalar_mul(rc1v(k, 0, TQ), resid[k][:], SR)
            sq = epool.tile([128, TQ], bf16, tag="e")
            nc.vector.tensor_mul(sq[:], resid[k][:], resid[k][:])
            sq1.append(sq)
        rows2o = rows2sp.tile([2, TQ], bf16, tag="r2s")
        rb_o = sbigS.tile([128, TQ], f32, tag="sbS")
        ln_rows(rc1v, sq1, TQ, srows, rows2o, rb_o,
                mean_c=1.0 / (C * SR), sq_c=1.0 / C,
                rb_bias=lncq_t[:])

        vt = [vpool.tile([128, 1040], bf16, tag="v", name=f"vt{i}") for i in range(NT)]
        for tt in range(NT):
            nc.gpsimd.memset(vt[tt][:, 64:1040:65], 1.0)
        dense_projV("wv", xbv, vt, corr=(aux_t["aux_v"], rows2), rstd_col=rcol)
        qT = [qpool.tile([128, TQ], bf16, tag="q", name=f"qT{i}") for i in range(NT)]

        def q_drain(mi, h, pt):
            nc.vector.tensor_mul(qT[mi][:], pt[:], rb_o[:])

        dense_proj_dr("wq", rc1pair, TQ, q_drain, corr=(aux_t["aux_q"], rows2o), wq_=nc.scalar)

        dump("d_qT", qT)
        dump("d_kT", kT)
        dump("d_v", vt)

        # =============== phase 4: self attention + interleaved cross-V ======
        v2t = [vpool.tile([128, 1040], bf16, tag="v", name=f"v2t{i}") for i in range(NT)]
        for tt in range(NT):
            nc.gpsimd.memset(v2t[tt][:, 64:1040:65], 1.0)
        oT = [opool.tile([128, TQ], bf16, tag="o", name=f"oT{i}") for i in range(NT)]
        attention(qT, kT, vt, oT, filler=make_cv_filler("wcv", fbv, v2t, bcv_t))
        dump("d_oT", oT)

        # =============== phase 6: self proj + residual (rc2 cast fused) ======
        rc2 = [rc8p.tile([128, 2 * TQ], fp8, tag="rc8", name=f"rc2_{i}")
               for i in range(4)]

        def rc2v(k, c0, c1):
            return rc2[k // 2][:, (k % 2) * TQ + c0:(k % 2) * TQ + c1]

        def rc2pair(kk, c0, c1):
            return rc2[kk][:].rearrange("p (j t) -> p j t", j=2)[:, :, c0:c1]

        def sp_drain(mi, h, pt):
            nc.vector.scalar_tensor_tensor(resid[mi][:], pt[:], bias_t["bsp"][:, mi:mi + 1],
                                           resid[mi][:], op0=AL.add, op1=AL.add)
            nc.vector.tensor_scalar_mul(rc2v(mi, 0, TQ), resid[mi][:], SR)

        dense_proj("wsp", oT, TQ, sp_drain)
        dump("d_r1", resid)

        # =============== phase 7: LN1 on updated own tokens ===============
        sq2 = []
        for k in range(NT):
            sq = epool.tile([128, TQ], bf16, tag="e")
            nc.vector.tensor_mul(sq[:], resid[k][:], resid[k][:])
            sq2.append(sq)
        rows2o2 = rows2sp.tile([2, TQ], bf16, tag="r2s")
        rb_o2 = sbigS.tile([128, TQ], f32, tag="sbS")
        ln_rows(rc2v, sq2, TQ, srows, rows2o2, rb_o2,
                mean_c=1.0 / (C * SR), sq_c=1.0 / C,
                rb_bias=lncq_t[:])

        # =============== phase 8: cross q ===============
        q2T = [qpool.tile([128, TQ], bf16, tag="q", name=f"q2T{i}") for i in range(NT)]

        def q2_drain(mi, h, pt):
            nc.vector.tensor_mul(q2T[mi][:], pt[:], rb_o2[:])

        dense_proj_dr("wcq", rc2pair, TQ, q2_drain, corr=(aux_t["aux_cq"], rows2o2),
                      wq_=nc.gpsimd)
        dump("d_q2T", q2T)

        # =============== phase 9: cross attention ===============
        o2T = [opool.tile([128, TQ], bf16, tag="o", name=f"o2T{i}") for i in range(NT)]
        ck_chunk(0)
        ck_chunk(1)
        attention(q2T, k2T, v2t, o2T, filler=ck_filler)
        dump("d_k2T", k2T)
        dump("d_o2T", o2T)

        # =============== phase 10: cross proj + residual (rc3 cast fused) ====
        rc3 = [rc8p.tile([128, 2 * TQ], fp8, tag="rc8", name=f"rc3_{i}")
               for i in range(4)]

        def rc3v(k, c0, c1):
            return rc3[k // 2][:, (k % 2) * TQ + c0:(k % 2) * TQ + c1]

        def cp_drain(mi, h, pt):
            nc.vector.scalar_tensor_tensor(resid[mi][:], pt[:], bias_t["bcp"][:, mi:mi + 1],
                                           resid[mi][:], op0=AL.add, op1=AL.add)
            nc.vector.tensor_scalar_mul(rc3v(mi, 0, TQ), resid[mi][:], SR)

        dense_proj("wcp", o2T, TQ, cp_drain, wq_=nc.gpsimd)
        dump("d_r2", resid)

        # =============== phase 11: LN2 (explicit normalize) ===============
        sq3 = []
        for k in range(NT):
            sq = epool.tile([128, TQ], bf16, tag="e")
            nc.vector.tensor_mul(sq[:], resid[k][:], resid[k][:])
            sq3.append(sq)
        rb_o3 = sbigS.tile([128, TQ], f32, tag="sbS")
        nmr_o3 = sbigS.tile([128, TQ], f32, tag="sbS")
        ln_rows(rc3v, sq3, TQ, srows, None, rb_o3, negmr_out=nmr_o3,
                mean_c=1.0 / (C * SR), sq_c=1.0 / C)
        # prime the gelu table after the last ln/exp rows, before fc drains
        nc.scalar.activation(dum[:], dum[:], AF.Gelu_apprx_tanh)
        ln2 = [qpool.tile([128, TQ], bf16, tag="q", name=f"ln2_{i}") for i in range(NT)]
        for k in range(NT):
            nc.vector.tensor_mul(ln2[k][:], resid[k][:], rb_o3[:])
            nc.vector.tensor_add(ln2[k][:], ln2[k][:], nmr_o3[:])

        dump("d_ln2", ln2)

        # =============== phase 12: MLP fc ===============
        m_sb = [actbig.tile([128, 1024], bf16, tag="actbig", name=f"m{i}")
                for i in range(16)]
        pcnt = 0
        fpools = ((pp, "pp"), (po, "po"))
        for g in range(8):
            wts = [wload("wfc", kk, 0, colbase=g * 512) for kk in range(4)]
            for ml in range(4):
                mi = g * 4 + ml
                pl, ptag = fpools[pcnt % 2]
                pcnt += 1
                pt = pl.tile([128, 512], f32, tag=ptag)
                for k in range(NT):
                    kk, j = k // 2, k % 2
                    nc.tensor.matmul(pt[:], wts[kk][:, j * 512 + ml * 128:j * 512 + (ml + 1) * 128],
                                     ln2[k][:], start=(k == 0), stop=(k == NT - 1))
                nc.scalar.activation(m_sb[mi // 2][:, (mi % 2) * 512:(mi % 2 + 1) * 512],
                                     pt[:], AF.Gelu_apprx_tanh,
                                     bias=bias_t["bfc"][:, mi:mi + 1])

        dump("d_m", m_sb)

        # =============== phase 13: MLP proj, k-outer over all 8 psum banks ===
        psA = ps.tile([128, 1024], f32, tag="ps")
        psB = ps.tile([128, 1024], f32, tag="ps")
        poA = po.tile([128, 512], f32, tag="po")
        poB = po.tile([128, 512], f32, tag="po")
        ppA = pp.tile([128, 512], f32, tag="pp")
        ppB = pp.tile([128, 512], f32, tag="pp")
        prq = [psA[:, 0:512], psA[:, 512:1024], psB[:, 0:512], psB[:, 512:1024],
               poA[:], poB[:], ppA[:], ppB[:]]
        for k in range(32):
            wt = wpool.tile([128, 1024], bf16, tag="wpool")
            (nc.gpsimd if k % 2 == 0 else nc.sync).dma_start(
                wt[:], w_d["wpr"][k * 128:(k + 1) * 128, :])
            ms = m_sb[k // 2][:, (k % 2) * 512:(k % 2 + 1) * 512]
            for j in range(8):
                nc.tensor.matmul(prq[j], wt[:, j * 128:(j + 1) * 128], ms,
                                 start=(k == 0), stop=(k == 31))
        outqs = [nc.gpsimd, nc.scalar, nc.sync]
        for j in range(8):
            nc.vector.scalar_tensor_tensor(resid[j][:], prq[j], bias_t["bpr"][:, j:j + 1],
                                           resid[j][:], op0=AL.add, op1=AL.add)
            outqs[j % 3].dma_start(outT_d[j * 128:(j + 1) * 128, :], resid[j][:])

    nc.compile()
    return nc


def _get_program():
    global _PROG
    if _PROG is None:
        _PROG = _build_program()
    return _PROG


def _prep_shared(inputs):
    g = {}

    def bf(a):
        return np.ascontiguousarray(np.asarray(a, dtype=np.float32)).astype(BF)

    def f(a):
        return np.ascontiguousarray(np.asarray(a, dtype=np.float32))

    def q8(a, s):
        return np.clip(np.asarray(a, np.float32) * s, -240.0, 240.0).astype(F8)

    inv = 1.0 / np.sqrt(DH)
    g1, b1 = np.asarray(inputs["ln1_g"]), np.asarray(inputs["ln1_b"])
    g2, b2 = np.asarray(inputs["ln2_g"]), np.asarray(inputs["ln2_b"])

    def fold(w, b, a, lb, ln=None):
        W = np.asarray(w, np.float32) + SCALE * (np.asarray(lb, np.float32)
                                                 @ np.asarray(a, np.float32))
        beff = np.asarray(b, np.float32).copy()
        if ln is not None:
            gg, bb = ln
            beff = beff + W @ bb
            W = W * gg[None, :]
        return W, beff

    def aux_of(WT, beff):
        # rows: colsum (for -m*r term), bias (times std term)
        return bf(np.stack([WT.sum(axis=0), beff], axis=0))

    Wqkv, bqkv = fold(inputs["sa_qkv_w"], inputs["sa_qkv_b"],
                      inputs["sa_qkv_a"], inputs["sa_qkv_lb"], ln=(g1, b1))
    qw, kw, vw = (Wqkv[i * C:(i + 1) * C] for i in range(3))
    qb, kb, vb = (bqkv[i * C:(i + 1) * C] for i in range(3))
    g["wq"] = q8(qw.T * inv, SW)
    g["wk"] = q8(kw.T, SW)
    g["wv"] = bf(vw.T)
    g["aux_q"] = aux_of((qw.T * inv) * (SW * SR), qb * inv * (SW * SR))
    g["aux_k"] = aux_of(kw.T * (SW * SX), kb * (SW * SX))
    g["aux_v"] = aux_of(vw.T * SX, vb * SX)

    Wsp, bsp = fold(inputs["sa_proj_w"], inputs["sa_proj_b"],
                    inputs["sa_proj_a"], inputs["sa_proj_lb"])
    g["wsp"] = bf(Wsp.T)
    g["bsp"] = f(bsp)

    Wcq, bcq = fold(inputs["ca_q_w"], inputs["ca_q_b"],
                    inputs["ca_q_a"], inputs["ca_q_lb"], ln=(g1, b1))
    g["wcq"] = q8(Wcq.T * inv, SW)
    g["aux_cq"] = aux_of((Wcq.T * inv) * (SW * SR), bcq * inv * (SW * SR))

    Wckv, bckv = fold(inputs["ca_kv_w"], inputs["ca_kv_b"],
                      inputs["ca_kv_a"], inputs["ca_kv_lb"])
    g["wck"] = q8(Wckv[0:C].T, SW)
    g["wcv"] = bf(Wckv[C:2 * C].T)
    g["bck"] = f(bckv[0:C])
    g["bcv_row"] = bf(bckv[C:2 * C].reshape(1, C) * SX)

    Wcp, bcp = fold(inputs["ca_proj_w"], inputs["ca_proj_b"],
                    inputs["ca_proj_a"], inputs["ca_proj_lb"])
    g["wcp"] = bf(Wcp.T)
    g["bcp"] = f(bcp)

    Wfc = np.asarray(inputs["fc_w"], np.float32) * g2[None, :]
    bfc = np.asarray(inputs["fc_b"], np.float32) + np.asarray(inputs["fc_w"], np.float32) @ b2
    g["wfc"] = bf(Wfc.T)
    g["bfc"] = f(bfc)
    g["wpr"] = bf(np.asarray(inputs["pr_w"]).T)
    g["bpr"] = f(inputs["pr_b"])
    return g


def _make_in_maps(inputs):
    inputs = {k: np.asarray(v) for k, v in inputs.items()}
    x, feat = inputs["x"], inputs["feature"]
    B = x.shape[0]
    shared = _prep_shared(inputs)

    bands = []
    for p in range(2):
        jj = np.arange(128).reshape(128, 1)
        ii = np.arange(64).reshape(1, 64)
        b = np.where(jj <= 2 * ii + p, 1.0, 0.0).astype(np.float32).astype(BF)
        bands.append(np.concatenate([b, b], axis=1))  # duplicated for 2-head strip

    in_maps = []
    xTs = [np.ascontiguousarray(np.asarray(x[b]).T, dtype=np.float32) for b in range(B)]
    fTs = [np.ascontiguousarray(np.asarray(feat[b]).T, dtype=np.float32) for b in range(B)]
    x8s = [np.clip(t * SX, -240, 240).astype(F8) for t in xTs]
    f8s = [np.clip(t * SX, -240, 240).astype(F8) for t in fTs]
    sqs = [(t * t).astype(BF) for t in xTs]
    for core in range(NCORES):
        b, p = core // 2, core % 2
        m = dict(shared)
        m["xbT"] = x8s[b]
        m["xqT"] = np.ascontiguousarray(xTs[b][:, p::2])
        m["fbT"] = f8s[b]
        m["sqxT"] = sqs[b]
        m["band"] = bands[p]
        in_maps.append(m)
    return in_maps, B


def kernel(**inputs):
    from concourse.bass_utils import run_bass_kernel_spmd

    nc = _get_program()
    in_maps, B = _make_in_maps(inputs)
    res = run_bass_kernel_spmd(nc, in_maps, core_ids=list(range(NCORES)))
    out = np.zeros((B, T, C), np.float32)
    for core in range(NCORES):
        b, p = core // 2, core % 2
        out[b, p::2, :] = np.asarray(res.results[core]["outT"], dtype=np.float32).T
    return out

